# revision 32
# baseline (speedup 1.0000x reference)
"""LASAGESConv GNN message-passing kernel for 8 Trainium2 NeuronCores.

Strategy (node-partitioned; host does layout/gather, device does all FLOPs):
- dst nodes split into 8 contiguous ranges (one per core); edges live with
  their dst core. Host pre-gathers edge rows feat16[src] into a per-core
  chunk-slot table streamed sequentially (big linear DMAs — no indirect DMA,
  whose per-instruction Pool desc-gen cost of ~1us/128 rows was the v1
  bottleneck at 97% Pool busy).
- Blocks of <=128 dst nodes; per (block, label) 3 chunks of 128 edge slots:
  each node's first two label-l edges sit at partition == node slot, so those
  chunks' one-hot is a constant identity matrix; only leftover edges need a
  val-driven is_equal one-hot (generated on Pool — SBUF-only there, since
  GPSIMD cannot access PSUM). Segment sums accumulate as 128-col fp16
  matmuls into a [128, 3*128] PSUM per block.
- MLP stage fused per 4-block group (512 node columns) in transposed layout
  [D, nodes]. Linearity: h_neigh = limlp_fr(s_fr + b*s_unk) +
  limlp_be(s_be + (1-b)*s_unk); bias terms split into constant parts (folded
  into Act eviction biases) + rank-1 b-dependent parts (one row-matmul each).
  Elementwise work balanced across DVE (gating, fused PSUM-bias+multiply via
  scalar_tensor_tensor), Act (activations, PSUM evictions) and Pool (one-hot
  gen, balance broadcast) to keep every engine under the PE roofline.
- fp16 streams/activations, fp32 PSUM accumulation, fp16 output (host widens).
"""

import numpy as np

_CACHE = {}

Q = 3                 # chunks per label per block
CPB = 3 * Q           # chunks per block
SG = 4                # blocks per MLP group


def _patch_tile_drain(tile, mybir, ScopedClock):
    """Walrus in this container rejects >2 sync waits on a Drain; split the
    Tile tail-drain waits onto individual NOPs."""
    if getattr(tile.TileContext, "_drain_patched", False):
        return

    def _drain_and_barrier(self, tick_clock, wait_clock):
        probe = self.nc.sync.nop(hint="tail_drain_waits", nofuse=True)
        wait_clock.add_sem_waits(
            probe.ins, ScopedClock({None: tick_clock.global_clock})
        )
        si = probe.ins.sync_info
        if si is not None and len(si.on_wait) > 1:
            waits = list(si.on_wait)
            del si.on_wait[1:]
            for w in waits[1:]:
                n = self.nc.sync.nop(hint="tail_drain_waits", nofuse=True)
                if n.ins.sync_info is None:
                    n.ins.sync_info = mybir.SyncInfo(on_wait=[w], on_update=[])
                else:
                    n.ins.sync_info.on_wait.append(w)
        self.nc.sync.drain()
        self.nc.all_engine_barrier()
        assert self.sems is not None
        popped = self.nc._tile_sem_poison_stack.pop()
        assert popped is self._sem_poison
        self.nc.clear_and_free_semaphores(list(self.sems.allocated().values()))
        self.nc.all_engine_barrier()

    tile.TileContext._drain_and_barrier = _drain_and_barrier
    tile.TileContext._drain_patched = True


def _split_sync_waits(nc, mybir, max_w=2):
    """Walrus codegen in this container bounds sync waits per instruction;
    move extra waits onto dedicated same-engine NOPs placed just before."""
    for bb in list(nc.main_func.blocks):
        new = []
        for ins in bb.instructions:
            si = ins.sync_info
            if si is not None and len(si.on_wait) > max_w:
                waits = list(si.on_wait)
                keep, move = waits[-max_w:], waits[:-max_w]
                del si.on_wait[:]
                si.on_wait.extend(keep)
                for w in move:
                    nop = nc.engines[ins.engine].nop(hint="wsplit", nofuse=True)
                    ni = nop.ins
                    nc.cur_bb.bb.instructions.remove(ni)
                    if ni.sync_info is None:
                        ni.sync_info = mybir.SyncInfo(on_wait=[w], on_update=[])
                    else:
                        ni.sync_info.on_wait.append(w)
                    new.append(ni)
            new.append(ins)
        bb.instructions[:] = new


def _build_program(N, D, NB):
    """Build the SPMD Bass program (same instruction stream on all 8 cores)."""
    import concourse.bass as bass
    import concourse.mybir as mybir
    import concourse.tile as tile
    from concourse.vector_clock import ScopedClock

    _patch_tile_drain(tile, mybir, ScopedClock)

    f16 = mybir.dt.float16
    f32 = mybir.dt.float32
    EQ = mybir.AluOpType.is_equal
    ADD = mybir.AluOpType.add
    MUL = mybir.AluOpType.mult
    SUB = mybir.AluOpType.subtract

    NG = NB // SG
    GC = SG * CPB               # chunks per group (36)
    C = NB * CPB                # total chunks

    nc = bass.Bass()
    gath_d = nc.dram_tensor("gathd", [128, C * 128], f16, kind="ExternalInput")
    hT_d = nc.dram_tensor("hT", [128, NB * 128], f16, kind="ExternalInput")
    val_d = nc.dram_tensor("val", [128, NB * 3], f32, kind="ExternalInput")
    ident_d = nc.dram_tensor("ident", [128, 128], f16, kind="ExternalInput")
    w_d = nc.dram_tensor("wcat", [128, 10 * 128 + 1], f16, kind="ExternalInput")
    b_d = nc.dram_tensor("bcat", [128, 9], f32, kind="ExternalInput")
    iota_d = nc.dram_tensor("iota128", [128, 128], f16, kind="ExternalInput")
    brows_d = nc.dram_tensor("brows", [1, 512], f16, kind="ExternalInput")
    out_d = nc.dram_tensor("outp", [128, NB * 128], f16, kind="ExternalOutput")

    W = {}
    wnames = ["frT1", "frT2", "beT1", "beT2", "frW1", "beW1",
              "frW2", "beW2", "selfW", "balW1"]

    with tile.TileContext(nc) as tc:
        with (
            tc.tile_pool(name="const", bufs=1) as cpool,
            tc.tile_pool(name="gath", bufs=3) as gpool,
            tc.tile_pool(name="oh", bufs=8) as ohpool,
            tc.tile_pool(name="sb", bufs=3) as spool,
            tc.tile_pool(name="mlp", bufs=3) as mpool,
            tc.tile_pool(name="ps_s", bufs=2, space="PSUM") as ps_s,
            tc.tile_pool(name="ps_g", bufs=4, space="PSUM") as ps_g,
            tc.tile_pool(name="ps_o", bufs=1, space="PSUM") as ps_o,
            tc.tile_pool(name="ps_b", bufs=1, space="PSUM") as ps_b,
        ):
            # ---- preload constants ----
            iota = cpool.tile([128, 128], f16, tag="iota")
            nc.sync.dma_start(out=iota[:], in_=iota_d[:])
            ident = cpool.tile([128, 128], f16, tag="ident")
            nc.sync.dma_start(out=ident[:], in_=ident_d[:])
            wcat = cpool.tile([128, 10 * 128 + 1], f16, tag="wcat")
            nc.sync.dma_start(out=wcat[:], in_=w_d[:])
            bcat = cpool.tile([128, 9], f32, tag="bcat")
            nc.sync.dma_start(out=bcat[:], in_=b_d[:])
            valt = cpool.tile([128, NB * 3], f32, tag="val")
            nc.sync.dma_start(out=valt[:], in_=val_d[:])
            brows = cpool.tile([1, 512], f16, tag="brows")
            nc.sync.dma_start(out=brows[:], in_=brows_d[:])

            for i, nm in enumerate(wnames):
                W[nm] = wcat[:, i * 128:(i + 1) * 128]
            balW2 = wcat[:, 10 * 128: 10 * 128 + 1]
            BIAS = {nm: bcat[:, i:i + 1] for i, nm in enumerate(
                ["frT1b", "frT2b", "beT1b", "beT2b", "ybfr", "ybbe",
                 "balb1", "finb"])}
            balb2 = bcat[0:1, 8:9]

            for g in range(NG):
                hT = mpool.tile([128, 512], f16, tag="hT")
                nc.sync.dma_start(out=hT[:], in_=hT_d[:, g * 512:(g + 1) * 512])

                # ---- balance = sigmoid(relu(h@W1+b1)@W2+b2) — emitted first:
                # it depends only on hT and gates the whole MLP stage ----
                pbal = ps_b.tile([128, 512], f32, tag="ps_b", space="PSUM")
                nc.tensor.matmul(out=pbal[:], lhsT=W["balW1"], rhs=hT[:],
                                 start=True, stop=True)
                a1 = mpool.tile([128, 512], f16, tag="a1")
                nc.scalar.activation(a1[:], pbal[:],
                                     mybir.ActivationFunctionType.Relu,
                                     bias=BIAS["balb1"])
                prow_t = ps_b.tile([128, 512], f32, tag="ps_b", space="PSUM")
                prow = prow_t[0:1, :]
                nc.tensor.matmul(out=prow, lhsT=balW2, rhs=a1[:],
                                 start=True, stop=True)
                brow = mpool.tile([1, 512], f16, tag="brow")
                nc.scalar.activation(brow[:], prow[:],
                                     mybir.ActivationFunctionType.Sigmoid,
                                     bias=balb2)
                bbc = mpool.tile([128, 512], f16, tag="bbc")
                nc.gpsimd.partition_broadcast(bbc[:], brow[:])

                gath = gpool.tile([128, GC, 128], f16, tag="gath")
                for hb in range(SG):
                    nc.sync.dma_start(
                        out=gath[:, hb * CPB:(hb + 1) * CPB, :].rearrange(
                            "p c e -> p (c e)"),
                        in_=gath_d[:, (g * SG + hb) * CPB * 128:
                                   (g * SG + hb + 1) * CPB * 128])

                # ---- label-pure segment sums per block ----
                # chunks j=0,1 hold each node's first two label-l edges at
                # partition == node slot, so their one-hot is the identity
                # (constant); only the leftover chunk j=2 needs a DVE one-hot.
                scat = spool.tile([128, 3 * 512], f16, tag="scat")
                for b in range(SG):
                    pss = ps_s.tile([128, 384], f32, tag="ps_s", space="PSUM")
                    for l in range(3):
                        for j in range(Q):
                            c = b * CPB + l * Q + j
                            if j < Q - 1:
                                rhs = ident[:]
                            else:
                                # SBUF-only op -> Pool engine (Act/DVE carry
                                # the PSUM evictions; GPSIMD can't touch PSUM)
                                oh = ohpool.tile([128, 128], f16, tag="oh")
                                nc.gpsimd.tensor_scalar(
                                    out=oh[:], in0=iota[:],
                                    scalar1=valt[:, (g * SG + b) * 3 + l:
                                                  (g * SG + b) * 3 + l + 1],
                                    scalar2=None, op0=EQ)
                                rhs = oh[:]
                            nc.tensor.matmul(
                                out=pss[:, l * 128:(l + 1) * 128],
                                lhsT=gath[:, c, :], rhs=rhs,
                                start=(j == 0), stop=(j == Q - 1))
                    # evict psum -> scat so each label becomes a contiguous
                    # [128, 512] region (split across Act and DVE)
                    src3 = pss[:].rearrange("p (l c) -> p l c", l=3)
                    dst3 = scat[:].rearrange("p (l b c) -> p l b c",
                                             l=3, b=SG)[:, :, b, :]
                    if b == 3:
                        nc.vector.tensor_copy(out=dst3, in_=src3)
                    else:
                        nc.scalar.copy(out=dst3, in_=src3)

                s_be = scat[:, 0:512]
                s_fr = scat[:, 512:1024]
                s_unk = scat[:, 1024:1536]

                # ---- u_fr = s_fr + b*s_unk ; u_be = (s_be+s_unk) - b*s_unk
                # (v = s_be+s_unk is independent of the balance chain) ----
                v = mpool.tile([128, 512], f16, tag="q")
                nc.vector.tensor_tensor(out=v[:], in0=s_be, in1=s_unk,
                                        op=ADD)
                tmp = mpool.tile([128, 512], f16, tag="tmp")
                nc.vector.tensor_tensor(out=tmp[:], in0=bbc[:], in1=s_unk,
                                        op=MUL)
                u_fr = mpool.tile([128, 512], f16, tag="u_fr")
                nc.vector.tensor_tensor(out=u_fr[:], in0=s_fr, in1=tmp[:],
                                        op=ADD)
                u_be = mpool.tile([128, 512], f16, tag="u_be")
                nc.vector.tensor_tensor(out=u_be[:], in0=v[:], in1=tmp[:],
                                        op=SUB)

                # ---- two gated MLP paths + self, accumulated in one PSUM ----
                # bias algebra: b1*(1+b) and b1*(2-b) split into a constant
                # part (folded into the y-eviction Act bias) and a rank-1
                # b-dependent part (one row-matmul against brow each);
                # likewise the pout b2 biases fold into finb + one row-matmul.
                pout = ps_o.tile([128, 512], f32, tag="ps_o", space="PSUM")
                BR = {"fr": brows[0:1, 0:128], "be": brows[0:1, 128:256]}
                for t, u in (("fr", u_fr), ("be", u_be)):
                    pg1 = ps_g.tile([128, 512], f32, tag="ps_g", space="PSUM")
                    nc.tensor.matmul(out=pg1[:], lhsT=W[t + "T1"], rhs=hT[:],
                                     start=True, stop=True)
                    x1 = mpool.tile([128, 512], f16, tag="x1")
                    nc.vector.scalar_tensor_tensor(
                        out=x1[:], in0=pg1[:], scalar=BIAS[t + "T1b"],
                        in1=u[:], op0=ADD, op1=MUL)
                    py = ps_g.tile([128, 512], f32, tag="ps_g", space="PSUM")
                    nc.tensor.matmul(out=py[:], lhsT=W[t + "W1"], rhs=x1[:],
                                     start=True, stop=False)
                    nc.tensor.matmul(out=py[:], lhsT=BR[t], rhs=brow[:],
                                     start=False, stop=True)
                    y = mpool.tile([128, 512], f16, tag="y")
                    nc.scalar.activation(y[:], py[:],
                                         mybir.ActivationFunctionType.Identity,
                                         bias=BIAS["yb" + t])
                    pg2 = ps_g.tile([128, 512], f32, tag="ps_g", space="PSUM")
                    nc.tensor.matmul(out=pg2[:], lhsT=W[t + "T2"], rhs=hT[:],
                                     start=True, stop=True)
                    x2 = mpool.tile([128, 512], f16, tag="x2")
                    nc.vector.scalar_tensor_tensor(
                        out=x2[:], in0=pg2[:], scalar=BIAS[t + "T2b"],
                        in1=y[:], op0=ADD, op1=MUL)
                    nc.tensor.matmul(out=pout[:], lhsT=W[t + "W2"], rhs=x2[:],
                                     start=(t == "fr"), stop=False)
                nc.tensor.matmul(out=pout[:], lhsT=brows[0:1, 256:384],
                                 rhs=brow[:], start=False, stop=False)
                nc.tensor.matmul(out=pout[:], lhsT=W["selfW"], rhs=hT[:],
                                 start=False, stop=True)
                res = mpool.tile([128, 512], f16, tag="res")
                nc.scalar.activation(res[:], pout[:],
                                     mybir.ActivationFunctionType.Relu,
                                     bias=BIAS["finb"])
                nc.sync.dma_start(out=out_d[:, g * 512:(g + 1) * 512],
                                  in_=res[:])
    _split_sync_waits(nc, mybir, 1)
    import bass_rust
    from concourse.library_config import all_libraries, standard
    lib_mask = {}
    for lib in all_libraries:
        for t in lib.instructions:
            lib_mask[t] = lib_mask.get(t, 0) | (1 << lib.index)
    bass_rust.insert_library_loads(nc, lib_mask, len(all_libraries),
                                  standard.index)
    mybir.codegen_inst_isa_subclasses(nc)
    return nc


def _pack(feat16, ds, ss, ls, core_lo, core_hi, NC, NLOC):
    """Per-core block packing + pre-gathered chunk tables.

    Per (block, label): each node's first two edges sit at partition ==
    node slot in chunks j=0,1 (identity one-hot on device); edges beyond
    the second go to the dynamic chunk j=2 with a val-driven one-hot.
    """
    blocks_all = []
    for c in range(NC):
        dsl = ds[core_lo[c]:core_hi[c]] - c * NLOC
        lsl = ls[core_lo[c]:core_hi[c]]
        cnt = np.zeros((NLOC, 3), np.int64)
        np.add.at(cnt, (dsl, lsl), 1)
        exc = np.maximum(cnt - (Q - 1), 0)
        cume = np.concatenate([np.zeros((1, 3), np.int64), np.cumsum(exc, 0)])
        blocks = []
        s = 0
        while s < NLOC:
            e = min(s + 128, NLOC)
            while e > s + 1 and (cume[e] - cume[s]).max() > 128:
                e -= 1
            blocks.append((s, e))
            s = e
        blocks_all.append(blocks)

    NB = max(len(b) for b in blocks_all)
    NB = ((NB + SG - 1) // SG) * SG

    gath_all, val_all, hT_all, vcols_all = [], [], [], []
    featT16 = np.ascontiguousarray(feat16.T)
    for c in range(NC):
        dsl = ds[core_lo[c]:core_hi[c]] - c * NLOC
        ssl = ss[core_lo[c]:core_hi[c]]
        lsl = ls[core_lo[c]:core_hi[c]]
        blocks = blocks_all[c]
        nb_of = np.zeros(NLOC, np.int64)
        bstart = np.zeros(NLOC, np.int64)
        for b, (s, e) in enumerate(blocks):
            nb_of[s:e] = b
            bstart[s:e] = s

        # rank of each edge within its (node, label) group
        o2 = np.argsort(dsl * 3 + lsl, kind="stable")
        d2, s2, l2 = dsl[o2], ssl[o2], lsl[o2]
        grp = d2 * 3 + l2
        newg = np.concatenate([[True], np.diff(grp) != 0])
        first = np.nonzero(newg)[0]
        gid = np.cumsum(newg) - 1
        r = np.arange(len(grp)) - first[gid]

        b2 = nb_of[d2]
        slot = d2 - bstart[d2]

        C = NB * CPB
        gath = np.zeros((128, C, 128), np.float16)
        val = np.full((128, NB * 3), 1000.0, np.float32)
        hT = np.zeros((128, NB * 128), np.float16)

        mc = r < (Q - 1)
        gath[slot[mc], b2[mc] * CPB + l2[mc] * Q + r[mc], :] = feat16[s2[mc]]

        md = ~mc
        keyd = b2[md] * 3 + l2[md]
        od = np.argsort(keyd, kind="stable")
        kd = keyd[od]
        newk = np.concatenate([[True], np.diff(kd) != 0])
        firstk = np.nonzero(newk)[0]
        kid = np.cumsum(newk) - 1
        pos = np.arange(len(kd)) - firstk[kid]
        dd, sd, ld, bd = (slot[md][od], s2[md][od], l2[md][od], b2[md][od])
        gath[pos, bd * CPB + ld * Q + (Q - 1), :] = feat16[sd]
        val[pos, bd * 3 + ld] = dd.astype(np.float32)

        vcols = []
        for b, (s, e) in enumerate(blocks):
            hT[:, b * 128: b * 128 + (e - s)] = \
                featT16[:, c * NLOC + s: c * NLOC + e]
            vcols.append(b * 128 + np.arange(e - s))
        gath_all.append(gath.reshape(128, C * 128))
        val_all.append(val)
        hT_all.append(hT)
        vcols_all.append(np.concatenate(vcols))
    return NB, gath_all, val_all, hT_all, vcols_all


def kernel(**inputs):
    inp = {k: np.asarray(v) for k, v in inputs.items()}
    feat = inp["feat"].astype(np.float32)
    src = inp["src"].astype(np.int64)
    dst = inp["dst"].astype(np.int64)
    labels = inp["labels"].astype(np.int64)
    N, D = feat.shape
    NC = 8
    assert N % NC == 0 and D == 128
    NLOC = N // NC

    lab = labels[src]
    order = np.argsort(dst, kind="stable")
    ds, ss, ls = dst[order], src[order], lab[order]
    core_lo = np.searchsorted(ds, np.arange(NC) * NLOC)
    core_hi = np.searchsorted(ds, (np.arange(NC) + 1) * NLOC)

    feat16 = feat.astype(np.float16)
    NB, gath_all, val_all, hT_all, vcols_all = _pack(
        feat16, ds, ss, ls, core_lo, core_hi, NC, NLOC)

    # weights: lhsT layout (pre-transposed), fp16
    wcat = np.concatenate([
        inp["fr_T1w"].T, inp["fr_T2w"].T, inp["be_T1w"].T, inp["be_T2w"].T,
        inp["fr_W1"].T, inp["be_W1"].T, inp["fr_W2"].T, inp["be_W2"].T,
        inp["self_W"].T, inp["bal_W1"].T, inp["bal_W2"].T,
    ], axis=1).astype(np.float16)
    # y-eviction biases carry the constant part of b1*(1+b) / b1*(2-b);
    # finb carries self_b plus the constant part of the pout b2 biases.
    bcat = np.zeros((128, 9), np.float32)
    for i, b in enumerate([
            inp["fr_T1b"], inp["fr_T2b"], inp["be_T1b"], inp["be_T2b"],
            inp["fr_b1"], 2.0 * inp["be_b1"], inp["bal_b1"],
            inp["self_b"] + inp["fr_b2"] + 2.0 * inp["be_b2"]]):
        bcat[:, i] = b
    bcat[0, 8] = float(inp["bal_b2"][0])
    iota128 = np.tile(np.arange(128, dtype=np.float16), (128, 1))
    brows = np.concatenate([inp["fr_b1"], -inp["be_b1"],
                            inp["fr_b2"] - inp["be_b2"],
                            np.zeros(128, np.float32)]
                           ).astype(np.float16)[None, :]

    key = (N, D, NB)
    if key not in _CACHE:
        _CACHE[key] = _build_program(N, D, NB)
    nc = _CACHE[key]

    from concourse.bass_utils import run_bass_kernel_spmd
    ident = np.eye(128, dtype=np.float16)
    in_maps = [{
        "gathd": gath_all[c], "hT": hT_all[c], "val": val_all[c],
        "wcat": wcat, "bcat": bcat, "iota128": iota128, "brows": brows,
        "ident": ident,
    } for c in range(NC)]
    res = run_bass_kernel_spmd(nc, in_maps, core_ids=list(range(NC)),
                               trace=False)

    out = np.empty((N, D), np.float32)
    for c in range(NC):
        out[c * NLOC:(c + 1) * NLOC] = \
            res.results[c]["outp"][:, vcols_all[c]].astype(np.float32).T
    return out
